# revision 2
# baseline (speedup 1.0000x reference)
"""Trainium2 Bass kernel for ConvMessageAggregator (fp16, DVE/ACT/Pool balanced).

Computes, for each node n (messages: [N, 16, 688] fp32):
  f1[i] = relu(w10*x[i] + w11*x[i+2] + b1)      i in 0..13   (dilated 2-tap conv)
  f2[i] = relu(w20*f1[i] + w21*f1[i+2] + b2)    i in 0..11
  out   = relu(sum_k mlp_w[k] * f2[6+k] + mlp_b)             -> [N, 688]

Only x rows 6..15 are consumed; the host stages those 10 rows as fp16
(rel err ~1e-3 vs the 2e-2 gate) and upcasts the fp16 result to fp32.

Engine plan (per conv: pre = p*(x_pv + r*x_ot + c), out = relu(p*pre)):
  DVE  A-pass: t = TSP(x_ot, mult r, add c)       4x mode, 0.27 ns/el
  DVE  B-pass: t += x_pv (TT add)                  2x mode, 0.54 ns/el
  ACT  C1: f1 = Relu(p1 * t)                       1x, bias folded via c
  ACT  C2[k!=root]: G_k = Relu(|w_k| p2 * u) = |w_k| relu(z_k)  (sign in tree)
  DVE  C2[root]: G = TSP(u, mult w_k*p2, max/min 0) = w_k relu(z_k)  signed
  Pool tree: 5 sign-merge TT add/sub on GpSimd (otherwise idle engine)
  DVE  final: out = TSP(T, add mlp_b, max 0)
Per-core loads ~ DVE 130us, ACT 128us, Pool 115us, DMA 95us (overlapped).
"""

import sys

for _p in ("/opt/trn_rl_repo",):
    if _p not in sys.path:
        sys.path.insert(0, _p)

import numpy as np

import concourse.bass as bass
import concourse.tile as tile
from concourse import mybir
from concourse.bass_utils import run_bass_kernel_spmd

N_FULL, L, MSG = 16384, 16, 688
N_CORES = 8
N_LOCAL = N_FULL // N_CORES  # 2048
P = 128
TW = 2                        # node blocks per tile
NTILES = N_LOCAL // (P * TW)  # 8
R0, NROWS = 6, 10

F16 = mybir.dt.float16
F32 = mybir.dt.float32
AF = mybir.ActivationFunctionType
OP = mybir.AluOpType


def _split_multi_waits(nc):
    """TPB instructions encode at most ONE semaphore wait; this walrus build's
    codegen rejects instructions with more. Hoist extra waits into standalone
    EventSemaphore ops on the same (in-order) sequencer."""
    for func in nc.m.functions:
        for bb in func.blocks:
            insts = list(bb.instructions)
            if not any(
                i.sync_info is not None and len(i.sync_info.on_wait) > 1
                for i in insts
            ):
                continue
            new = []
            for inst in insts:
                si = inst.sync_info
                if si is not None and len(si.on_wait) > 1:
                    waits = list(si.on_wait)
                    for j, w in enumerate(waits[:-1]):
                        new.append(
                            mybir.InstEventSemaphore(
                                name=f"{inst.name}-hoistw{j}",
                                engine=inst.engine,
                                sync_info=mybir.SyncInfo(on_wait=[w], on_update=[]),
                            )
                        )
                    inst.sync_info = mybir.SyncInfo(
                        on_wait=[waits[-1]], on_update=list(si.on_update)
                    )
                new.append(inst)
            bb.instructions = new


def _conv_split(wa, wb, b):
    """Factor pre[i] = wa*in[i] + wb*in[i+2] + b as p*(in[pv] + r*in[ot] + c)
    with |r| <= 1 (pv = dominant tap), p = dominant weight, c = b/p."""
    if abs(wa) >= abs(wb):
        p, r, pv, ot = wa, (wb / wa if wa != 0.0 else 0.0), 0, 2
    else:
        p, r, pv, ot = wb, wa / wb, 2, 0
    c = b / p if p != 0.0 else 0.0
    return p, r, c, pv, ot


def build_program(w10, w11, b1, w20, w21, b2, mlp_w, mlp_b):
    nc = bass.Bass(trn_type="TRN2", name="conv_msg_agg")
    x = nc.dram_tensor("x", [N_LOCAL, NROWS, MSG], F16, kind="ExternalInput")
    out = nc.dram_tensor("out", [N_LOCAL, MSG], F16, kind="ExternalOutput")

    p1, r1, c1, pv1, ot1 = _conv_split(w10, w11, b1)
    p2, r2, c2, pv2, ot2 = _conv_split(w20, w21, b2)
    nzk = [k for k in range(6) if mlp_w[k] != 0.0]
    # the DVE-routed (signed) root row: largest |w_k| for stability
    k_root = max(nzk, key=lambda k: abs(mlp_w[k])) if nzk else None

    with tile.TileContext(nc) as tc:
        with (
            tc.tile_pool(name="xin", bufs=3) as pool_x,
            tc.tile_pool(name="v1p", bufs=2) as pool_1,
            tc.tile_pool(name="v2p", bufs=2) as pool_2,
            tc.tile_pool(name="outp", bufs=3) as pool_o,
        ):
            for it in range(NTILES):
                n0 = it * TW * P
                xt = pool_x.tile([P, NROWS, TW, MSG], F16, tag="x")
                front_split = it <= 1
                back_split = it == NTILES - 1
                blks = (
                    [(blk, blk + 1) for blk in range(TW)]
                    if front_split
                    else [(0, TW)]
                )
                bblks = (
                    [(blk, blk + 1) for blk in range(TW)]
                    if back_split
                    else [(0, TW)]
                )
                for lo, hi in blks:
                    nc.sync.dma_start(
                        out=xt[:, :, lo:hi, :],
                        in_=x[n0 + lo * P : n0 + hi * P].rearrange(
                            "(b p) r m -> p r b m", b=hi - lo
                        ),
                    )

                # conv1: t = r1*x_ot + c1 (TSP), t += x_pv (TT),
                #        f1 = Relu(p1*t) (ACT, in place)
                v1 = pool_1.tile([P, 8, TW, MSG], F16, tag="v1")
                if p1 == 0.0:
                    nc.vector.memset(v1[:], max(b1, 0.0))
                else:
                    for lo, hi in blks:
                        nc.vector.tensor_scalar(
                            out=v1[:, :, lo:hi, :],
                            in0=xt[:, ot1 : ot1 + 8, lo:hi, :],
                            scalar1=r1,
                            scalar2=c1,
                            op0=OP.mult,
                            op1=OP.add,
                        )
                        nc.vector.tensor_tensor(
                            out=v1[:, :, lo:hi, :],
                            in0=v1[:, :, lo:hi, :],
                            in1=xt[:, pv1 : pv1 + 8, lo:hi, :],
                            op=OP.add,
                        )
                        nc.scalar.activation(
                            out=v1[:, :, lo:hi, :], in_=v1[:, :, lo:hi, :],
                            func=AF.Relu, bias=0.0, scale=p1,
                        )

                # conv2: u = r2*f1_ot + c2 (TSP), u += f1_pv (TT)
                v2 = pool_2.tile([P, 6, TW, MSG], F16, tag="v2")
                ot = pool_o.tile([P, TW, MSG], F16, tag="o")
                for lo, hi in bblks:
                    if p2 == 0.0 or not nzk:
                        base = max(b2, 0.0) if p2 == 0.0 else 0.0
                        tval = sum(mlp_w[k] * base for k in nzk) + mlp_b
                        if p2 == 0.0 or tval <= 0.0:
                            nc.vector.memset(ot[:, lo:hi, :], max(tval, 0.0))
                            nc.sync.dma_start(
                                out=out[n0 + lo * P : n0 + hi * P].rearrange(
                                    "(b p) m -> p b m", b=hi - lo
                                ),
                                in_=ot[:, lo:hi, :],
                            )
                            continue
                    nc.vector.tensor_scalar(
                        out=v2[:, :, lo:hi, :],
                        in0=v1[:, ot2 : ot2 + 6, lo:hi, :],
                        scalar1=r2,
                        scalar2=c2,
                        op0=OP.mult,
                        op1=OP.add,
                    )
                    nc.vector.tensor_tensor(
                        out=v2[:, :, lo:hi, :],
                        in0=v2[:, :, lo:hi, :],
                        in1=v1[:, pv2 : pv2 + 6, lo:hi, :],
                        op=OP.add,
                    )

                    # C2 passes: root row signed on DVE, rest on ACT
                    terms = []  # (sign, row_ap); root first so it can be the
                    # positive merge target (its value is already signed)
                    for k in nzk:
                        dst = v2[:, k, lo:hi, :]
                        if k == k_root:
                            nc.vector.tensor_scalar(
                                out=dst,
                                in0=dst,
                                scalar1=mlp_w[k] * p2,
                                scalar2=0.0,
                                op0=OP.mult,
                                op1=OP.max if mlp_w[k] > 0 else OP.min,
                            )
                            terms.insert(0, (1, dst))
                        else:
                            nc.scalar.activation(
                                out=dst, in_=dst, func=AF.Relu,
                                bias=0.0, scale=abs(mlp_w[k]) * p2,
                            )
                            terms.append((1 if mlp_w[k] > 0 else -1, dst))

                    # pairwise sign-merge tree on GpSimd (Pool engine)
                    while len(terms) > 1:
                        pos = [t for t in terms if t[0] > 0]
                        neg = [t for t in terms if t[0] < 0]
                        if len(neg) >= 2:
                            (sa, aa), (sb, ab) = neg[0], neg[1]
                            op = OP.add
                        elif len(pos) >= 2 and len(neg) == 0:
                            (sa, aa), (sb, ab) = pos[0], pos[1]
                            op = OP.add
                        elif pos and neg:  # one neg left: fold into a pos
                            (sa, aa), (sb, ab) = pos[0], neg[0]
                            op = OP.subtract
                        else:
                            (sa, aa), (sb, ab) = pos[0], pos[1]
                            op = OP.add
                        nc.gpsimd.tensor_tensor(out=aa, in0=aa, in1=ab, op=op)
                        terms = [
                            t for t in terms if t[1] is not aa and t[1] is not ab
                        ]
                        terms.insert(0, (sa, aa))

                    # final: out = Relu(T + mlp_b) on DVE (root-first merge
                    # order guarantees the survivor carries sign +1)
                    assert terms[0][0] > 0
                    nc.vector.tensor_scalar(
                        out=ot[:, lo:hi, :],
                        in0=terms[0][1],
                        scalar1=mlp_b,
                        scalar2=0.0,
                        op0=OP.add,
                        op1=OP.max,
                    )
                    nc.sync.dma_start(
                        out=out[n0 + lo * P : n0 + hi * P].rearrange(
                            "(b p) m -> p b m", b=hi - lo
                        ),
                        in_=ot[:, lo:hi, :],
                    )
    _split_multi_waits(nc)
    return nc


def run(inputs, trace=False, **spmd_kwargs):
    """Build + run on 8 cores. Returns (full_output, BassKernelResults)."""
    msgs = np.asarray(inputs["messages"])
    assert msgs.shape == (N_FULL, L, MSG), msgs.shape
    xs = np.ascontiguousarray(msgs[:, R0 : R0 + NROWS, :], dtype=np.float16)

    c1w = np.asarray(inputs["conv1_w"], dtype=np.float64)
    c2w = np.asarray(inputs["conv2_w"], dtype=np.float64)
    mlw = np.asarray(inputs["mlp_w"], dtype=np.float64)
    nc = build_program(
        float(c1w[0]),
        float(c1w[1]),
        float(np.asarray(inputs["conv1_b"], dtype=np.float64)),
        float(c2w[0]),
        float(c2w[1]),
        float(np.asarray(inputs["conv2_b"], dtype=np.float64)),
        [float(v) for v in mlw],
        float(np.asarray(inputs["mlp_b"], dtype=np.float64)),
    )

    in_maps = [
        {"x": xs[i * N_LOCAL : (i + 1) * N_LOCAL]} for i in range(N_CORES)
    ]
    res = run_bass_kernel_spmd(
        nc, in_maps, core_ids=list(range(N_CORES)), trace=trace, **spmd_kwargs
    )
    full = np.concatenate([r["out"] for r in res.results], axis=0).astype(
        np.float32
    )
    return full, res


def kernel(**inputs) -> np.ndarray:
    return run(inputs, trace=False)[0]


# revision 3
# speedup vs baseline: 1.4146x; 1.4146x over previous
"""Trainium2 Bass kernel for ConvMessageAggregator (fp16, DVE/ACT/Pool balanced).

Computes, for each node n (messages: [N, 16, 688] fp32):
  f1[i] = relu(w10*x[i] + w11*x[i+2] + b1)      i in 0..13   (dilated 2-tap conv)
  f2[i] = relu(w20*f1[i] + w21*f1[i+2] + b2)    i in 0..11
  out   = relu(sum_k mlp_w[k] * f2[6+k] + mlp_b)             -> [N, 688]

Only x rows 6..15 are consumed; the host stages those 10 rows as fp16
(rel err ~1e-3 vs the 2e-2 gate) and upcasts the fp16 result to fp32.

Engine plan (per conv: pre = p*(x_pv + r*x_ot + c), out = relu(p*pre)):
  DVE  A-pass: t = TSP(x_ot, mult r, add c)       4x mode, 0.27 ns/el
  DVE  B-pass: t += x_pv (TT add)                  2x mode, 0.54 ns/el
  ACT  C1: f1 = Relu(p1 * t)                       1x, bias folded via c
  ACT  C2[k!=root]: G_k = Relu(|w_k| p2 * u) = |w_k| relu(z_k)  (sign in tree)
  DVE  C2[root]: G = TSP(u, mult w_k*p2, max/min 0) = w_k relu(z_k)  signed
  Pool tree: 5 sign-merge TT add/sub on GpSimd (otherwise idle engine)
  DVE  final: out = TSP(T, add mlp_b, max 0)
Per-core loads ~ DVE 130us, ACT 128us, Pool 115us, DMA 95us (overlapped).
"""

import sys

for _p in ("/opt/trn_rl_repo",):
    if _p not in sys.path:
        sys.path.insert(0, _p)

import numpy as np

import concourse.bass as bass
import concourse.tile as tile
from concourse import mybir
from concourse.bass_utils import run_bass_kernel_spmd

N_FULL, L, MSG = 16384, 16, 688
N_CORES = 8
N_LOCAL = N_FULL // N_CORES  # 2048
P = 128
TW = 2                        # node blocks per tile
NTILES = N_LOCAL // (P * TW)  # 8
R0, NROWS = 6, 10

F16 = mybir.dt.float16
F32 = mybir.dt.float32
AF = mybir.ActivationFunctionType
OP = mybir.AluOpType


def _split_multi_waits(nc):
    """TPB instructions encode at most ONE semaphore wait; this walrus build's
    codegen rejects instructions with more. Hoist extra waits into standalone
    EventSemaphore ops on the same (in-order) sequencer."""
    for func in nc.m.functions:
        for bb in func.blocks:
            insts = list(bb.instructions)
            if not any(
                i.sync_info is not None and len(i.sync_info.on_wait) > 1
                for i in insts
            ):
                continue
            new = []
            for inst in insts:
                si = inst.sync_info
                if si is not None and len(si.on_wait) > 1:
                    waits = list(si.on_wait)
                    for j, w in enumerate(waits[:-1]):
                        new.append(
                            mybir.InstEventSemaphore(
                                name=f"{inst.name}-hoistw{j}",
                                engine=inst.engine,
                                sync_info=mybir.SyncInfo(on_wait=[w], on_update=[]),
                            )
                        )
                    inst.sync_info = mybir.SyncInfo(
                        on_wait=[waits[-1]], on_update=list(si.on_update)
                    )
                new.append(inst)
            bb.instructions = new


def _conv_split(wa, wb, b):
    """Factor pre[i] = wa*in[i] + wb*in[i+2] + b as p*(in[pv] + r*in[ot] + c)
    with |r| <= 1 (pv = dominant tap), p = dominant weight, c = b/p."""
    if abs(wa) >= abs(wb):
        p, r, pv, ot = wa, (wb / wa if wa != 0.0 else 0.0), 0, 2
    else:
        p, r, pv, ot = wb, wa / wb, 2, 0
    c = b / p if p != 0.0 else 0.0
    return p, r, c, pv, ot


def build_program(w10, w11, b1, w20, w21, b2, mlp_w, mlp_b):
    nc = bass.Bass(trn_type="TRN2", name="conv_msg_agg")
    x = nc.dram_tensor("x", [N_LOCAL, NROWS, MSG], F16, kind="ExternalInput")
    out = nc.dram_tensor("out", [N_LOCAL, MSG], F16, kind="ExternalOutput")

    p1, r1, c1, pv1, ot1 = _conv_split(w10, w11, b1)
    p2, r2, c2, pv2, ot2 = _conv_split(w20, w21, b2)
    nzk = [k for k in range(6) if mlp_w[k] != 0.0]
    # the DVE-routed (signed) root row: largest |w_k| for stability
    k_root = max(nzk, key=lambda k: abs(mlp_w[k])) if nzk else None

    with tile.TileContext(nc) as tc:
        with (
            tc.tile_pool(name="xin", bufs=3) as pool_x,
            tc.tile_pool(name="v1p", bufs=2) as pool_1,
            tc.tile_pool(name="v2p", bufs=2) as pool_2,
            tc.tile_pool(name="outp", bufs=3) as pool_o,
        ):
            for it in range(NTILES):
                n0 = it * TW * P
                xt = pool_x.tile([P, NROWS, TW, MSG], F16, tag="x")
                front_split = it <= 1
                back_split = it == NTILES - 1
                blks = (
                    [(blk, blk + 1) for blk in range(TW)]
                    if front_split
                    else [(0, TW)]
                )
                bblks = (
                    [(blk, blk + 1) for blk in range(TW)]
                    if back_split
                    else [(0, TW)]
                )
                for lo, hi in blks:
                    nc.sync.dma_start(
                        out=xt[:, :, lo:hi, :],
                        in_=x[n0 + lo * P : n0 + hi * P].rearrange(
                            "(b p) r m -> p r b m", b=hi - lo
                        ),
                    )

                # conv1: t = r1*x_ot + c1 (TSP), t += x_pv (TT),
                #        f1 = Relu(p1*t) (ACT, in place)
                v1 = pool_1.tile([P, 8, TW, MSG], F16, tag="v1")
                if p1 == 0.0:
                    nc.vector.memset(v1[:], max(b1, 0.0))
                else:
                    for lo, hi in blks:
                        nc.vector.tensor_scalar(
                            out=v1[:, :, lo:hi, :],
                            in0=xt[:, ot1 : ot1 + 8, lo:hi, :],
                            scalar1=r1,
                            scalar2=c1,
                            op0=OP.mult,
                            op1=OP.add,
                        )
                        nc.vector.tensor_tensor(
                            out=v1[:, :, lo:hi, :],
                            in0=v1[:, :, lo:hi, :],
                            in1=xt[:, pv1 : pv1 + 8, lo:hi, :],
                            op=OP.add,
                        )
                        nc.scalar.activation(
                            out=v1[:, :, lo:hi, :], in_=v1[:, :, lo:hi, :],
                            func=AF.Relu, bias=0.0, scale=p1,
                        )

                # conv2: u = r2*f1_ot + c2 (TSP), u += f1_pv (TT)
                v2 = pool_2.tile([P, 6, TW, MSG], F16, tag="v2")
                ot = pool_o.tile([P, TW, MSG], F16, tag="o")
                for lo, hi in bblks:
                    if p2 == 0.0 or not nzk:
                        base = max(b2, 0.0) if p2 == 0.0 else 0.0
                        tval = sum(mlp_w[k] * base for k in nzk) + mlp_b
                        if p2 == 0.0 or tval <= 0.0:
                            nc.vector.memset(ot[:, lo:hi, :], max(tval, 0.0))
                            nc.sync.dma_start(
                                out=out[n0 + lo * P : n0 + hi * P].rearrange(
                                    "(b p) m -> p b m", b=hi - lo
                                ),
                                in_=ot[:, lo:hi, :],
                            )
                            continue
                    nc.vector.tensor_scalar(
                        out=v2[:, :, lo:hi, :],
                        in0=v1[:, ot2 : ot2 + 6, lo:hi, :],
                        scalar1=r2,
                        scalar2=c2,
                        op0=OP.mult,
                        op1=OP.add,
                    )
                    nc.vector.tensor_tensor(
                        out=v2[:, :, lo:hi, :],
                        in0=v2[:, :, lo:hi, :],
                        in1=v1[:, pv2 : pv2 + 6, lo:hi, :],
                        op=OP.add,
                    )

                    # C2 passes on ACT (|w_k|-scaled; signs resolved in the
                    # tree). If no positive w_k exists, the largest-|w| row
                    # goes to DVE signed so the tree survivor is always +.
                    any_pos = any(mlp_w[k] > 0 for k in nzk)
                    terms = []  # (sign, row_ap)
                    for k in nzk:
                        dst = v2[:, k, lo:hi, :]
                        if not any_pos and k == k_root:
                            nc.vector.tensor_scalar(
                                out=dst,
                                in0=dst,
                                scalar1=mlp_w[k] * p2,
                                scalar2=0.0,
                                op0=OP.mult,
                                op1=OP.max if mlp_w[k] > 0 else OP.min,
                            )
                            terms.insert(0, (1, dst))
                        else:
                            nc.scalar.activation(
                                out=dst, in_=dst, func=AF.Relu,
                                bias=0.0, scale=abs(mlp_w[k]) * p2,
                            )
                            terms.append((1 if mlp_w[k] > 0 else -1, dst))

                    # pairwise sign-merge tree (TT add/sub on DVE)
                    while len(terms) > 1:
                        pos = [t for t in terms if t[0] > 0]
                        neg = [t for t in terms if t[0] < 0]
                        if len(neg) >= 2:
                            (sa, aa), (sb, ab) = neg[0], neg[1]
                            op = OP.add
                        elif len(pos) >= 2 and len(neg) == 0:
                            (sa, aa), (sb, ab) = pos[0], pos[1]
                            op = OP.add
                        else:  # one neg left: fold into a pos
                            (sa, aa), (sb, ab) = pos[0], neg[0]
                            op = OP.subtract
                        nc.vector.tensor_tensor(out=aa, in0=aa, in1=ab, op=op)
                        terms = [
                            t for t in terms if t[1] is not aa and t[1] is not ab
                        ]
                        terms.insert(0, (sa, aa))

                    # final: out = Relu(T + mlp_b) on DVE (root-first merge
                    # order guarantees the survivor carries sign +1)
                    assert terms[0][0] > 0
                    nc.vector.tensor_scalar(
                        out=ot[:, lo:hi, :],
                        in0=terms[0][1],
                        scalar1=mlp_b,
                        scalar2=0.0,
                        op0=OP.add,
                        op1=OP.max,
                    )
                    nc.sync.dma_start(
                        out=out[n0 + lo * P : n0 + hi * P].rearrange(
                            "(b p) m -> p b m", b=hi - lo
                        ),
                        in_=ot[:, lo:hi, :],
                    )
    _split_multi_waits(nc)
    return nc


def run(inputs, trace=False, **spmd_kwargs):
    """Build + run on 8 cores. Returns (full_output, BassKernelResults)."""
    msgs = np.asarray(inputs["messages"])
    assert msgs.shape == (N_FULL, L, MSG), msgs.shape
    xs = np.ascontiguousarray(msgs[:, R0 : R0 + NROWS, :], dtype=np.float16)

    c1w = np.asarray(inputs["conv1_w"], dtype=np.float64)
    c2w = np.asarray(inputs["conv2_w"], dtype=np.float64)
    mlw = np.asarray(inputs["mlp_w"], dtype=np.float64)
    nc = build_program(
        float(c1w[0]),
        float(c1w[1]),
        float(np.asarray(inputs["conv1_b"], dtype=np.float64)),
        float(c2w[0]),
        float(c2w[1]),
        float(np.asarray(inputs["conv2_b"], dtype=np.float64)),
        [float(v) for v in mlw],
        float(np.asarray(inputs["mlp_b"], dtype=np.float64)),
    )

    in_maps = [
        {"x": xs[i * N_LOCAL : (i + 1) * N_LOCAL]} for i in range(N_CORES)
    ]
    res = run_bass_kernel_spmd(
        nc, in_maps, core_ids=list(range(N_CORES)), trace=trace, **spmd_kwargs
    )
    full = np.concatenate([r["out"] for r in res.results], axis=0).astype(
        np.float32
    )
    return full, res


def kernel(**inputs) -> np.ndarray:
    return run(inputs, trace=False)[0]
